# revision 7
# baseline (speedup 1.0000x reference)
"""Fused KV-cache attention block for Trainium2, tensor-parallel over 8 NeuronCores.

Model (per reference): x:[2,1024,2048]; Q/K/V = x@W^T+b (nn.Linear), 16 heads x 128;
K/V concatenated with a 3072-token cache; softmax attention; output projection.
Returns (out, K, V).

Sharding: 2 heads per core (column-parallel W_Q/W_K/W_V, row-parallel W_O).
Each core computes its heads' QKV projections, flash-style attention over the
4096-key sequence, and a partial output projection. Host sums the 8 partial
outputs and assembles the returned K/V caches (past cache rows pass through
on the host; only the 1024 new rows come from the device).

On-device layouts keep every matmul in the engine-native [contraction-on-
partitions] form with zero on-device transposes:
  - x is fed transposed (x^T chunks), prepared host-side
  - Q^T/K^T [dk, tok] come out of the projections directly
  - scores are computed transposed, S^T[k, q] = K @ Q^T
  - softmax denominators via a ones-column matmul; normalization is a
    per-q (free-axis) multiply with exp(-ln(sum)) broadcast over partitions
All matmuls run bf16 with fp32 PSUM accumulation.
"""

import sys

sys.path.insert(0, "/opt/trn_rl_repo")

import numpy as np
import ml_dtypes

import concourse.bass as bass  # noqa: F401  (registers AP machinery)
import concourse.mybir as mybir
import concourse.tile as tile
from concourse import bacc
from concourse.bass_utils import run_bass_kernel_spmd

F32 = mybir.dt.float32
BF16 = mybir.dt.bfloat16
AF = mybir.ActivationFunctionType

B, S, D, H, DK, PAST = 2, 1024, 2048, 16, 128, 3072
T = B * S                  # 2048 tokens
NCORES = 8
HL = H // NCORES           # 2 heads per core
E = HL * DK                # 256 per-core projection width
NCH_D = D // 128           # 16 contraction chunks over d_model
PCH = PAST // 128          # 24 past k-chunks
SCH = S // 128             # 8 new k-chunks per batch
KCH = PCH + SCH            # 32 k-chunks per (b, head)
NB = 512                   # matmul free-dim block
QB = S // NB               # 2 q-blocks per batch

_BF = ml_dtypes.bfloat16
_CACHE = {}


def _build():
    nc = bacc.Bacc("TRN2", target_bir_lowering=False, debug=False)

    x_t = nc.dram_tensor("x_t", [128, NCH_D * T], BF16, kind="ExternalInput")
    wq_t = nc.dram_tensor("wq_t", [128, NCH_D * E], BF16, kind="ExternalInput")
    wk_t = nc.dram_tensor("wk_t", [128, NCH_D * E], BF16, kind="ExternalInput")
    wv_t = nc.dram_tensor("wv_t", [128, NCH_D * E], BF16, kind="ExternalInput")
    wo_t = nc.dram_tensor("wo_t", [128, HL * D], BF16, kind="ExternalInput")
    bq2 = nc.dram_tensor("bq2", [128, HL], F32, kind="ExternalInput")
    bk2 = nc.dram_tensor("bk2", [128, HL], F32, kind="ExternalInput")
    bv_row = nc.dram_tensor("bv_row", [1, E], BF16, kind="ExternalInput")
    ones_c = nc.dram_tensor("ones_c", [128, 1], BF16, kind="ExternalInput")
    ones_r = nc.dram_tensor("ones_r", [1, 128], BF16, kind="ExternalInput")
    past_kt = nc.dram_tensor("past_kt", [B * HL, 128, PAST], BF16, kind="ExternalInput")
    past_v = nc.dram_tensor("past_v", [B * HL, 128, PAST], BF16, kind="ExternalInput")

    out_p = nc.dram_tensor("out_p", [T, D], F32, kind="ExternalOutput")
    k_new_t = nc.dram_tensor("k_new_t", [E, T], F32, kind="ExternalOutput")
    v_new = nc.dram_tensor("v_new", [T, E], F32, kind="ExternalOutput")

    with tile.TileContext(nc) as tc:
        with tc.tile_pool(name="big", bufs=1) as big, \
             tc.tile_pool(name="work", bufs=1) as work:
            # ---- resident SBUF tensors ----
            x_sb = big.tile([128, NCH_D * T], BF16)       # x^T chunks
            wq_sb = big.tile([128, NCH_D * E], BF16)
            wk_sb = big.tile([128, NCH_D * E], BF16)
            wv_sb = big.tile([128, NCH_D * E], BF16)
            wo_sb = big.tile([128, HL * D], BF16)
            bq_sb = big.tile([128, HL], F32)
            bk_sb = big.tile([128, HL], F32)
            bvr_sb = big.tile([1, E], BF16)
            onesc_sb = big.tile([128, 1], BF16)
            onesr_sb = big.tile([1, 128], BF16)
            qT_sb = big.tile([128, HL * T], BF16)         # Q^T (scaled, biased)
            kT_sb = big.tile([128, HL * T], BF16)         # K_new^T (biased)
            v_sb = big.tile([128, SCH * B * E], BF16)     # V_new tiles [tok128, t*E+e]
            attnT_sb = big.tile([128, HL * T], BF16)      # attention output^T

            nc.sync.dma_start(wq_sb[:], wq_t.ap())
            # x chunk-wise, spread over engines' DMA queues so the first
            # projection matmuls start early
            dma_engs = (nc.sync, nc.gpsimd, nc.scalar)
            for c in range(NCH_D):
                dma_engs[c % 3].dma_start(x_sb[:, c * T:(c + 1) * T],
                                          x_t.ap()[:, c * T:(c + 1) * T])
            nc.sync.dma_start(wk_sb[:], wk_t.ap())
            nc.sync.dma_start(wv_sb[:], wv_t.ap())
            nc.sync.dma_start(wo_sb[:], wo_t.ap())
            nc.sync.dma_start(bq_sb[:], bq2.ap())
            nc.sync.dma_start(bk_sb[:], bk2.ap())
            nc.sync.dma_start(bvr_sb[:], bv_row.ap())
            nc.sync.dma_start(onesc_sb[:], ones_c.ap())
            nc.sync.dma_start(onesr_sb[:], ones_r.ap())

            # PSUM tags (one pool, 8 banks total): mm512 x2, small x2,
            # sT x2, pv x2
            psum_cm = tc.tile_pool(name="psum", bufs=1, space="PSUM")
            psum = psum_cm.__enter__()

            # ---- phase A: projections ----
            for w_sb, b_sb, dstT, is_k in ((wq_sb, bq_sb, qT_sb, False),
                                           (wk_sb, bk_sb, kT_sb, True)):
                for et in range(HL):
                    for nb in range(T // NB):
                        ps = psum.tile([128, NB], F32, tag="mm512", bufs=2)
                        for c in range(NCH_D):
                            nc.tensor.matmul(
                                ps[:],
                                w_sb[:, c * E + et * 128:c * E + et * 128 + 128],
                                x_sb[:, c * T + nb * NB:c * T + (nb + 1) * NB],
                                start=(c == 0), stop=(c == NCH_D - 1))
                        col = et * T + nb * NB
                        if is_k:
                            kf = work.tile([128, NB], F32, tag="kf32", bufs=2)
                            nc.scalar.activation(kf[:], ps[:], AF.Identity,
                                                 bias=b_sb[:, et:et + 1])
                            nc.vector.tensor_copy(dstT[:, col:col + NB], kf[:])
                            nc.sync.dma_start(
                                k_new_t.ap()[et * 128:(et + 1) * 128,
                                             nb * NB:(nb + 1) * NB], kf[:])
                        else:
                            nc.scalar.activation(dstT[:, col:col + NB], ps[:],
                                                 AF.Identity, bias=b_sb[:, et:et + 1])

            # V: bias seeded by a rank-1 ones @ bv matmul
            for t in range(T // 128):
                ps = psum.tile([128, E], F32, tag="small", bufs=2)
                nc.tensor.matmul(ps[:], onesr_sb[:], bvr_sb[:],
                                 start=True, stop=False)
                for c in range(NCH_D):
                    nc.tensor.matmul(
                        ps[:],
                        x_sb[:, c * T + t * 128:c * T + t * 128 + 128],
                        wv_sb[:, c * E:(c + 1) * E],
                        start=False, stop=(c == NCH_D - 1))
                nc.scalar.activation(v_sb[:, t * E:(t + 1) * E], ps[:], AF.Copy)
                vf = work.tile([128, E], F32, tag="vf32", bufs=2)
                nc.vector.tensor_copy(vf[:], ps[:])
                nc.sync.dma_start(v_new.ap()[t * 128:(t + 1) * 128, :], vf[:])

            # ---- phase B: attention, 8 subunits (batch, head, q-block) ----
            def chunk_aps(b, h, j, pastk, pastv):
                if j < PCH:
                    return (pastk[:, j * 128:(j + 1) * 128],
                            pastv[:, j * 128:(j + 1) * 128])
                jj = j - PCH
                return (kT_sb[:, h * T + b * S + jj * 128:
                              h * T + b * S + jj * 128 + 128],
                        v_sb[:, (b * SCH + jj) * E + h * 128:
                             (b * SCH + jj) * E + h * 128 + 128])

            # W_o partial projection for a token-tile range; interleaved
            # per batch right after that batch's attention units finish so
            # the PE never idles across the phase boundary
            def emit_wo(t_lo, t_hi):
                for t in range(t_lo, t_hi):
                    for nb in range(D // NB):
                        ps = psum.tile([128, NB], F32, tag="mm512", bufs=2,
                                       name=f"wo_ps_{t}_{nb}")
                        for h2 in range(HL):
                            nc.tensor.matmul(
                                ps[:],
                                attnT_sb[:, h2 * T + t * 128:h2 * T + t * 128 + 128],
                                wo_sb[:, h2 * D + nb * NB:h2 * D + (nb + 1) * NB],
                                start=(h2 == 0), stop=(h2 == HL - 1))
                        osb = work.tile([128, NB], F32, tag="osb", bufs=4,
                                        name=f"osb_{t}_{nb}")
                        nc.scalar.copy(osb[:], ps[:])
                        nc.sync.dma_start(
                            out_p.ap()[t * 128:(t + 1) * 128,
                                       nb * NB:(nb + 1) * NB],
                            osb[:])

            for u in range(B * HL):
                b, h = u // HL, u % HL
                pastk = work.tile([128, PAST], BF16, tag="pastk", bufs=2)
                nc.sync.dma_start(pastk[:], past_kt.ap()[u])
                pastv = work.tile([128, PAST], BF16, tag="pastv", bufs=2)
                nc.sync.dma_start(pastv[:], past_v.ap()[u])
                for qb in range(QB):
                    q0 = b * S + qb * NB
                    q_ap = qT_sb[:, h * T + q0:h * T + q0 + NB]
                    pv = psum.tile([128, NB], F32, tag="pv", bufs=2)
                    sums = psum.tile([1, NB], F32, tag="small", bufs=2)
                    for j in range(KCH):
                        kT_ap, v_ap = chunk_aps(b, h, j, pastk, pastv)
                        sT = psum.tile([128, NB], F32, tag="sT", bufs=2)
                        nc.tensor.matmul(sT[:], kT_ap, q_ap, start=True, stop=True)
                        pT = work.tile([128, NB], BF16, tag="pT", bufs=4)
                        nc.scalar.activation(pT[:], sT[:], AF.Exp)
                        nc.tensor.matmul(pv[:], v_ap, pT[:],
                                         start=(j == 0), stop=(j == KCH - 1))
                        nc.tensor.matmul(sums[:], onesc_sb[:], pT[:],
                                         start=(j == 0), stop=(j == KCH - 1))
                    # drain accumulators fast; normalize off the critical path
                    araw = work.tile([128, NB], F32, tag="araw", bufs=2)
                    nc.vector.tensor_copy(araw[:], pv[:])
                    recip = work.tile([1, NB], F32, tag="recip", bufs=2)
                    nc.vector.reciprocal(recip[:], sums[:])
                    recip_b = work.tile([128, NB], F32, tag="recip_b", bufs=2)
                    nc.gpsimd.partition_broadcast(recip_b[:], recip[:])
                    nc.vector.tensor_mul(attnT_sb[:, h * T + q0:h * T + q0 + NB],
                                         araw[:], recip_b[:])
                if u % HL == HL - 1:
                    emit_wo(b * (T // 256), (b + 1) * (T // 256))

            psum_cm.__exit__(None, None, None)

    nc.compile()
    return nc


def _prep_inputs(x, past_key, past_value, Wq, bq, Wk, bk, Wv, bv, Wo, bo):
    """Build the 8 per-core input maps (all arrays pre-arranged to SBUF layouts)."""
    scl = np.float32(1.0 / np.sqrt(DK))
    xf = np.asarray(x, np.float32).reshape(T, D)
    # x^T chunks: [p, c*T + t] = x[t, c*128+p]
    x_t = np.ascontiguousarray(xf.reshape(T, NCH_D, 128).transpose(2, 1, 0)
                               ).reshape(128, NCH_D * T).astype(_BF)
    ones_c = np.ones((128, 1), _BF)
    ones_r = np.ones((1, 128), _BF)

    def w_chunks(Wslice):  # [E, D] -> W^T chunk layout [128, NCH_D*E]
        wt = np.asarray(Wslice, np.float32).T  # [D, E]
        return np.ascontiguousarray(wt.reshape(NCH_D, 128, E).transpose(1, 0, 2)
                                    ).reshape(128, NCH_D * E).astype(_BF)

    in_maps = []
    for c in range(NCORES):
        r0, r1 = c * E, (c + 1) * E
        wo_c = np.asarray(Wo, np.float32)[:, r0:r1].T  # [E, D]
        wo_t = np.ascontiguousarray(wo_c.reshape(HL, 128, D).transpose(1, 0, 2)
                                    ).reshape(128, HL * D).astype(_BF)
        pk = np.asarray(past_key, np.float32)[:, c * HL:(c + 1) * HL]   # [B,HL,PAST,DK]
        pkt = np.ascontiguousarray(pk.transpose(0, 1, 3, 2)
                                   ).reshape(B * HL, 128, PAST).astype(_BF)
        pvv = np.asarray(past_value, np.float32)[:, c * HL:(c + 1) * HL]
        pvt = np.ascontiguousarray(
            pvv.reshape(B, HL, PCH, 128, DK).transpose(0, 1, 3, 2, 4)
        ).reshape(B * HL, 128, PAST).astype(_BF)
        in_maps.append({
            "x_t": x_t,
            "wq_t": w_chunks(np.asarray(Wq, np.float32)[r0:r1] * scl),
            "wk_t": w_chunks(np.asarray(Wk)[r0:r1]),
            "wv_t": w_chunks(np.asarray(Wv)[r0:r1]),
            "wo_t": wo_t,
            "bq2": np.ascontiguousarray(
                (np.asarray(bq, np.float32)[r0:r1] * scl).reshape(HL, 128).T),
            "bk2": np.ascontiguousarray(
                np.asarray(bk, np.float32)[r0:r1].reshape(HL, 128).T),
            "bv_row": np.asarray(bv, np.float32)[r0:r1].reshape(1, E).astype(_BF),
            "ones_c": ones_c,
            "ones_r": ones_r,
            "past_kt": pkt,
            "past_v": pvt,
        })
    return in_maps


def _assemble(results, past_key, past_value, bo):
    out = np.zeros((T, D), np.float32)
    for c in range(NCORES):
        out += results[c]["out_p"]
    out += np.asarray(bo, np.float32)
    out = out.reshape(B, S, D)

    K = np.empty((B, H, PAST + S, DK), np.float32)
    V = np.empty((B, H, PAST + S, DK), np.float32)
    K[:, :, :PAST] = np.asarray(past_key, np.float32)
    V[:, :, :PAST] = np.asarray(past_value, np.float32)
    for c in range(NCORES):
        knt = results[c]["k_new_t"].reshape(HL, 128, B, S).transpose(2, 0, 3, 1)
        K[:, c * HL:(c + 1) * HL, PAST:] = knt
        vnw = results[c]["v_new"].reshape(B, S, HL, DK).transpose(0, 2, 1, 3)
        V[:, c * HL:(c + 1) * HL, PAST:] = vnw
    return out, K, V


def run(inputs, trace=False):
    if "nc" not in _CACHE:
        _CACHE["nc"] = _build()
    nc = _CACHE["nc"]
    in_maps = _prep_inputs(**inputs)
    res = run_bass_kernel_spmd(nc, in_maps, core_ids=list(range(NCORES)),
                               trace=trace)
    out, K, V = _assemble(res.results, inputs["past_key"], inputs["past_value"],
                          inputs["bo"])
    return (out, K, V), res


def kernel(**inputs):
    (out, K, V), _ = run(inputs)
    return out, K, V


# revision 8
# speedup vs baseline: 1.0146x; 1.0146x over previous
"""Fused KV-cache attention block for Trainium2, tensor-parallel over 8 NeuronCores.

Model (per reference): x:[2,1024,2048]; Q/K/V = x@W^T+b (nn.Linear), 16 heads x 128;
K/V concatenated with a 3072-token cache; softmax attention; output projection.
Returns (out, K, V).

Sharding: 2 heads per core (column-parallel W_Q/W_K/W_V, row-parallel W_O).
Each core computes its heads' QKV projections, flash-style attention over the
4096-key sequence, and a partial output projection. Host sums the 8 partial
outputs and assembles the returned K/V caches (past cache rows pass through
on the host; only the 1024 new rows come from the device).

On-device layouts keep every matmul in the engine-native [contraction-on-
partitions] form with zero on-device transposes:
  - x is fed transposed (x^T chunks), prepared host-side
  - Q^T/K^T [dk, tok] come out of the projections directly
  - scores are computed transposed, S^T[k, q] = K @ Q^T
  - softmax denominators via a ones-column matmul; normalization is a
    per-q (free-axis) multiply with exp(-ln(sum)) broadcast over partitions
All matmuls run bf16 with fp32 PSUM accumulation.
"""

import sys

sys.path.insert(0, "/opt/trn_rl_repo")

import numpy as np
import ml_dtypes

import concourse.bass as bass  # noqa: F401  (registers AP machinery)
import concourse.mybir as mybir
import concourse.tile as tile
from concourse import bacc
from concourse.bass_utils import run_bass_kernel_spmd

F32 = mybir.dt.float32
BF16 = mybir.dt.bfloat16
AF = mybir.ActivationFunctionType

B, S, D, H, DK, PAST = 2, 1024, 2048, 16, 128, 3072
T = B * S                  # 2048 tokens
NCORES = 8
HL = H // NCORES           # 2 heads per core
E = HL * DK                # 256 per-core projection width
NCH_D = D // 128           # 16 contraction chunks over d_model
PCH = PAST // 128          # 24 past k-chunks
SCH = S // 128             # 8 new k-chunks per batch
KCH = PCH + SCH            # 32 k-chunks per (b, head)
NB = 512                   # matmul free-dim block
QB = S // NB               # 2 q-blocks per batch

_BF = ml_dtypes.bfloat16
_CACHE = {}


def _build():
    nc = bacc.Bacc("TRN2", target_bir_lowering=False, debug=False)

    x_t = nc.dram_tensor("x_t", [128, NCH_D * T], BF16, kind="ExternalInput")
    wq_t = nc.dram_tensor("wq_t", [128, NCH_D * E], BF16, kind="ExternalInput")
    wk_t = nc.dram_tensor("wk_t", [128, NCH_D * E], BF16, kind="ExternalInput")
    wv_t = nc.dram_tensor("wv_t", [128, NCH_D * E], BF16, kind="ExternalInput")
    wo_t = nc.dram_tensor("wo_t", [128, HL * D], BF16, kind="ExternalInput")
    bq2 = nc.dram_tensor("bq2", [128, HL], F32, kind="ExternalInput")
    bk2 = nc.dram_tensor("bk2", [128, HL], F32, kind="ExternalInput")
    bv_row = nc.dram_tensor("bv_row", [1, E], BF16, kind="ExternalInput")
    ones_c = nc.dram_tensor("ones_c", [128, 1], BF16, kind="ExternalInput")
    ones_r = nc.dram_tensor("ones_r", [1, 128], BF16, kind="ExternalInput")
    past_kt = nc.dram_tensor("past_kt", [B * HL, 128, PAST], BF16, kind="ExternalInput")
    past_v = nc.dram_tensor("past_v", [B * HL, 128, PAST], BF16, kind="ExternalInput")

    out_p = nc.dram_tensor("out_p", [T, D], F32, kind="ExternalOutput")
    k_new_t = nc.dram_tensor("k_new_t", [E, T], F32, kind="ExternalOutput")
    v_new = nc.dram_tensor("v_new", [T, E], F32, kind="ExternalOutput")

    with tile.TileContext(nc) as tc:
        with tc.tile_pool(name="big", bufs=1) as big, \
             tc.tile_pool(name="work", bufs=1) as work:
            # ---- resident SBUF tensors ----
            x_sb = big.tile([128, NCH_D * T], BF16)       # x^T chunks
            wq_sb = big.tile([128, NCH_D * E], BF16)
            wk_sb = big.tile([128, NCH_D * E], BF16)
            wv_sb = big.tile([128, NCH_D * E], BF16)
            wo_sb = big.tile([128, HL * D], BF16)
            bq_sb = big.tile([128, HL], F32)
            bk_sb = big.tile([128, HL], F32)
            bvr_sb = big.tile([1, E], BF16)
            onesc_sb = big.tile([128, 1], BF16)
            onesr_sb = big.tile([1, 128], BF16)
            qT_sb = big.tile([128, HL * T], BF16)         # Q^T (scaled, biased)
            kT_sb = big.tile([128, HL * T], BF16)         # K_new^T (biased)
            v_sb = big.tile([128, SCH * B * E], BF16)     # V_new tiles [tok128, t*E+e]
            attnT_sb = big.tile([128, HL * T], BF16)      # attention output^T

            nc.sync.dma_start(wq_sb[:], wq_t.ap())
            # x chunk-wise, spread over engines' DMA queues so the first
            # projection matmuls start early
            dma_engs = (nc.sync, nc.gpsimd, nc.scalar)
            for c in range(NCH_D):
                dma_engs[c % 3].dma_start(x_sb[:, c * T:(c + 1) * T],
                                          x_t.ap()[:, c * T:(c + 1) * T])
            nc.sync.dma_start(wk_sb[:], wk_t.ap())
            nc.sync.dma_start(wv_sb[:], wv_t.ap())
            nc.sync.dma_start(wo_sb[:], wo_t.ap())
            nc.sync.dma_start(bq_sb[:], bq2.ap())
            nc.sync.dma_start(bk_sb[:], bk2.ap())
            nc.sync.dma_start(bvr_sb[:], bv_row.ap())
            nc.sync.dma_start(onesc_sb[:], ones_c.ap())
            nc.sync.dma_start(onesr_sb[:], ones_r.ap())

            # PSUM tags (one pool, 8 banks total): mm512 x2, small x2,
            # sT x2, pv x2
            psum_cm = tc.tile_pool(name="psum", bufs=1, space="PSUM")
            psum = psum_cm.__enter__()

            # ---- phase A: projections ----
            for w_sb, b_sb, dstT, is_k in ((wq_sb, bq_sb, qT_sb, False),
                                           (wk_sb, bk_sb, kT_sb, True)):
                for et in range(HL):
                    for nb in range(T // NB):
                        ps = psum.tile([128, NB], F32, tag="mm512", bufs=2)
                        for c in range(NCH_D):
                            nc.tensor.matmul(
                                ps[:],
                                w_sb[:, c * E + et * 128:c * E + et * 128 + 128],
                                x_sb[:, c * T + nb * NB:c * T + (nb + 1) * NB],
                                start=(c == 0), stop=(c == NCH_D - 1))
                        col = et * T + nb * NB
                        if is_k:
                            kf = work.tile([128, NB], F32, tag="kf32", bufs=2)
                            nc.scalar.activation(kf[:], ps[:], AF.Identity,
                                                 bias=b_sb[:, et:et + 1])
                            nc.vector.tensor_copy(dstT[:, col:col + NB], kf[:])
                            nc.sync.dma_start(
                                k_new_t.ap()[et * 128:(et + 1) * 128,
                                             nb * NB:(nb + 1) * NB], kf[:])
                        else:
                            nc.scalar.activation(dstT[:, col:col + NB], ps[:],
                                                 AF.Identity, bias=b_sb[:, et:et + 1])

            # V: bias seeded by a rank-1 ones @ bv matmul
            for t in range(T // 128):
                ps = psum.tile([128, E], F32, tag="small", bufs=2)
                nc.tensor.matmul(ps[:], onesr_sb[:], bvr_sb[:],
                                 start=True, stop=False)
                for c in range(NCH_D):
                    nc.tensor.matmul(
                        ps[:],
                        x_sb[:, c * T + t * 128:c * T + t * 128 + 128],
                        wv_sb[:, c * E:(c + 1) * E],
                        start=False, stop=(c == NCH_D - 1))
                nc.scalar.activation(v_sb[:, t * E:(t + 1) * E], ps[:], AF.Copy)
                vf = work.tile([128, E], F32, tag="vf32", bufs=2)
                nc.vector.tensor_copy(vf[:], ps[:])
                nc.sync.dma_start(v_new.ap()[t * 128:(t + 1) * 128, :], vf[:])

            # ---- phase B: attention, 8 subunits (batch, head, q-block) ----
            def chunk_aps(b, h, j, pastk, pastv):
                if j < PCH:
                    return (pastk[:, j * 128:(j + 1) * 128],
                            pastv[:, j * 128:(j + 1) * 128])
                jj = j - PCH
                return (kT_sb[:, h * T + b * S + jj * 128:
                              h * T + b * S + jj * 128 + 128],
                        v_sb[:, (b * SCH + jj) * E + h * 128:
                             (b * SCH + jj) * E + h * 128 + 128])

            for u in range(B * HL):
                b, h = u // HL, u % HL
                pastk = work.tile([128, PAST], BF16, tag="pastk", bufs=2)
                nc.sync.dma_start(pastk[:], past_kt.ap()[u])
                pastv = work.tile([128, PAST], BF16, tag="pastv", bufs=2)
                nc.sync.dma_start(pastv[:], past_v.ap()[u])
                for qb in range(QB):
                    q0 = b * S + qb * NB
                    q_ap = qT_sb[:, h * T + q0:h * T + q0 + NB]
                    pv = psum.tile([128, NB], F32, tag="pv", bufs=2)
                    sums = psum.tile([1, NB], F32, tag="small", bufs=2)
                    for j in range(KCH):
                        kT_ap, v_ap = chunk_aps(b, h, j, pastk, pastv)
                        sT = psum.tile([128, NB], F32, tag="sT", bufs=2)
                        nc.tensor.matmul(sT[:], kT_ap, q_ap, start=True, stop=True)
                        pT = work.tile([128, NB], BF16, tag="pT", bufs=4)
                        nc.scalar.activation(pT[:], sT[:], AF.Exp)
                        nc.tensor.matmul(pv[:], v_ap, pT[:],
                                         start=(j == 0), stop=(j == KCH - 1))
                        nc.tensor.matmul(sums[:], onesc_sb[:], pT[:],
                                         start=(j == 0), stop=(j == KCH - 1))
                    # drain accumulators fast; normalize off the critical path
                    araw = work.tile([128, NB], F32, tag="araw", bufs=2)
                    nc.vector.tensor_copy(araw[:], pv[:])
                    recip = work.tile([1, NB], F32, tag="recip", bufs=2)
                    nc.vector.reciprocal(recip[:], sums[:])
                    recip_b = work.tile([128, NB], F32, tag="recip_b", bufs=2)
                    nc.gpsimd.partition_broadcast(recip_b[:], recip[:])
                    nc.vector.tensor_mul(attnT_sb[:, h * T + q0:h * T + q0 + NB],
                                         araw[:], recip_b[:])

            # ---- phase C: output projection (row-parallel partial) ----
            for t in range(T // 128):
                for nb in range(D // NB):
                    ps = psum.tile([128, NB], F32, tag="mm512", bufs=2)
                    for h in range(HL):
                        nc.tensor.matmul(
                            ps[:],
                            attnT_sb[:, h * T + t * 128:h * T + t * 128 + 128],
                            wo_sb[:, h * D + nb * NB:h * D + (nb + 1) * NB],
                            start=(h == 0), stop=(h == HL - 1))
                    osb = work.tile([128, NB], F32, tag="osb", bufs=4)
                    nc.scalar.copy(osb[:], ps[:])
                    nc.sync.dma_start(
                        out_p.ap()[t * 128:(t + 1) * 128, nb * NB:(nb + 1) * NB],
                        osb[:])
            psum_cm.__exit__(None, None, None)

    nc.compile()
    return nc


def _prep_inputs(x, past_key, past_value, Wq, bq, Wk, bk, Wv, bv, Wo, bo):
    """Build the 8 per-core input maps (all arrays pre-arranged to SBUF layouts)."""
    scl = np.float32(1.0 / np.sqrt(DK))
    xf = np.asarray(x, np.float32).reshape(T, D)
    # x^T chunks: [p, c*T + t] = x[t, c*128+p]
    x_t = np.ascontiguousarray(xf.reshape(T, NCH_D, 128).transpose(2, 1, 0)
                               ).reshape(128, NCH_D * T).astype(_BF)
    ones_c = np.ones((128, 1), _BF)
    ones_r = np.ones((1, 128), _BF)

    def w_chunks(Wslice):  # [E, D] -> W^T chunk layout [128, NCH_D*E]
        wt = np.asarray(Wslice, np.float32).T  # [D, E]
        return np.ascontiguousarray(wt.reshape(NCH_D, 128, E).transpose(1, 0, 2)
                                    ).reshape(128, NCH_D * E).astype(_BF)

    in_maps = []
    for c in range(NCORES):
        r0, r1 = c * E, (c + 1) * E
        wo_c = np.asarray(Wo, np.float32)[:, r0:r1].T  # [E, D]
        wo_t = np.ascontiguousarray(wo_c.reshape(HL, 128, D).transpose(1, 0, 2)
                                    ).reshape(128, HL * D).astype(_BF)
        pk = np.asarray(past_key, np.float32)[:, c * HL:(c + 1) * HL]   # [B,HL,PAST,DK]
        pkt = np.ascontiguousarray(pk.transpose(0, 1, 3, 2)
                                   ).reshape(B * HL, 128, PAST).astype(_BF)
        pvv = np.asarray(past_value, np.float32)[:, c * HL:(c + 1) * HL]
        pvt = np.ascontiguousarray(
            pvv.reshape(B, HL, PCH, 128, DK).transpose(0, 1, 3, 2, 4)
        ).reshape(B * HL, 128, PAST).astype(_BF)
        in_maps.append({
            "x_t": x_t,
            "wq_t": w_chunks(np.asarray(Wq, np.float32)[r0:r1] * scl),
            "wk_t": w_chunks(np.asarray(Wk)[r0:r1]),
            "wv_t": w_chunks(np.asarray(Wv)[r0:r1]),
            "wo_t": wo_t,
            "bq2": np.ascontiguousarray(
                (np.asarray(bq, np.float32)[r0:r1] * scl).reshape(HL, 128).T),
            "bk2": np.ascontiguousarray(
                np.asarray(bk, np.float32)[r0:r1].reshape(HL, 128).T),
            "bv_row": np.asarray(bv, np.float32)[r0:r1].reshape(1, E).astype(_BF),
            "ones_c": ones_c,
            "ones_r": ones_r,
            "past_kt": pkt,
            "past_v": pvt,
        })
    return in_maps


def _assemble(results, past_key, past_value, bo):
    out = np.zeros((T, D), np.float32)
    for c in range(NCORES):
        out += results[c]["out_p"]
    out += np.asarray(bo, np.float32)
    out = out.reshape(B, S, D)

    K = np.empty((B, H, PAST + S, DK), np.float32)
    V = np.empty((B, H, PAST + S, DK), np.float32)
    K[:, :, :PAST] = np.asarray(past_key, np.float32)
    V[:, :, :PAST] = np.asarray(past_value, np.float32)
    for c in range(NCORES):
        knt = results[c]["k_new_t"].reshape(HL, 128, B, S).transpose(2, 0, 3, 1)
        K[:, c * HL:(c + 1) * HL, PAST:] = knt
        vnw = results[c]["v_new"].reshape(B, S, HL, DK).transpose(0, 2, 1, 3)
        V[:, c * HL:(c + 1) * HL, PAST:] = vnw
    return out, K, V


def run(inputs, trace=False):
    if "nc" not in _CACHE:
        _CACHE["nc"] = _build()
    nc = _CACHE["nc"]
    in_maps = _prep_inputs(**inputs)
    res = run_bass_kernel_spmd(nc, in_maps, core_ids=list(range(NCORES)),
                               trace=trace)
    out, K, V = _assemble(res.results, inputs["past_key"], inputs["past_value"],
                          inputs["bo"])
    return (out, K, V), res


def kernel(**inputs):
    (out, K, V), _ = run(inputs)
    return out, K, V


# revision 9
# speedup vs baseline: 1.0174x; 1.0027x over previous
"""Fused KV-cache attention block for Trainium2, tensor-parallel over 8 NeuronCores.

Model (per reference): x:[2,1024,2048]; Q/K/V = x@W^T+b (nn.Linear), 16 heads x 128;
K/V concatenated with a 3072-token cache; softmax attention; output projection.
Returns (out, K, V).

Sharding: 2 heads per core (column-parallel W_Q/W_K/W_V, row-parallel W_O).
Each core computes its heads' QKV projections, flash-style attention over the
4096-key sequence, and a partial output projection. Host sums the 8 partial
outputs and assembles the returned K/V caches (past cache rows pass through
on the host; only the 1024 new rows come from the device).

On-device layouts keep every matmul in the engine-native [contraction-on-
partitions] form with zero on-device transposes:
  - x is fed transposed (x^T chunks), prepared host-side
  - Q^T/K^T [dk, tok] come out of the projections directly
  - scores are computed transposed, S^T[k, q] = K @ Q^T
  - softmax denominators via a ones-column matmul; normalization is a
    per-q (free-axis) multiply with exp(-ln(sum)) broadcast over partitions
All matmuls run bf16 with fp32 PSUM accumulation.
"""

import sys

sys.path.insert(0, "/opt/trn_rl_repo")

import numpy as np
import ml_dtypes

import concourse.bass as bass  # noqa: F401  (registers AP machinery)
import concourse.mybir as mybir
import concourse.tile as tile
from concourse import bacc
from concourse.bass_utils import run_bass_kernel_spmd

F32 = mybir.dt.float32
BF16 = mybir.dt.bfloat16
AF = mybir.ActivationFunctionType

B, S, D, H, DK, PAST = 2, 1024, 2048, 16, 128, 3072
T = B * S                  # 2048 tokens
NCORES = 8
HL = H // NCORES           # 2 heads per core
E = HL * DK                # 256 per-core projection width
NCH_D = D // 128           # 16 contraction chunks over d_model
PCH = PAST // 128          # 24 past k-chunks
SCH = S // 128             # 8 new k-chunks per batch
KCH = PCH + SCH            # 32 k-chunks per (b, head)
NB = 512                   # matmul free-dim block
QB = S // NB               # 2 q-blocks per batch

_BF = ml_dtypes.bfloat16
_CACHE = {}


def _build():
    nc = bacc.Bacc("TRN2", target_bir_lowering=False, debug=False)

    x_t = nc.dram_tensor("x_t", [128, NCH_D * T], BF16, kind="ExternalInput")
    wq_t = nc.dram_tensor("wq_t", [128, NCH_D * E], BF16, kind="ExternalInput")
    wk_t = nc.dram_tensor("wk_t", [128, NCH_D * E], BF16, kind="ExternalInput")
    wv_t = nc.dram_tensor("wv_t", [128, NCH_D * E], BF16, kind="ExternalInput")
    wo_t = nc.dram_tensor("wo_t", [128, HL * D], BF16, kind="ExternalInput")
    bq2 = nc.dram_tensor("bq2", [128, HL], F32, kind="ExternalInput")
    bk2 = nc.dram_tensor("bk2", [128, HL], F32, kind="ExternalInput")
    bv_row = nc.dram_tensor("bv_row", [1, E], BF16, kind="ExternalInput")
    ones_c = nc.dram_tensor("ones_c", [128, 1], BF16, kind="ExternalInput")
    ones_r = nc.dram_tensor("ones_r", [1, 128], BF16, kind="ExternalInput")
    past_kt = nc.dram_tensor("past_kt", [B * HL, 128, PAST], BF16, kind="ExternalInput")
    past_v = nc.dram_tensor("past_v", [B * HL, 128, PAST], BF16, kind="ExternalInput")

    out_p = nc.dram_tensor("out_p", [T, D], F32, kind="ExternalOutput")
    k_new_t = nc.dram_tensor("k_new_t", [E, T], F32, kind="ExternalOutput")
    v_new = nc.dram_tensor("v_new", [T, E], F32, kind="ExternalOutput")

    with tile.TileContext(nc) as tc:
        with tc.tile_pool(name="big", bufs=1) as big, \
             tc.tile_pool(name="work", bufs=1) as work:
            # ---- resident SBUF tensors ----
            x_sb = big.tile([128, NCH_D * T], BF16)       # x^T chunks
            wq_sb = big.tile([128, NCH_D * E], BF16)
            wk_sb = big.tile([128, NCH_D * E], BF16)
            wv_sb = big.tile([128, NCH_D * E], BF16)
            wo_sb = big.tile([128, HL * D], BF16)
            bq_sb = big.tile([128, HL], F32)
            bk_sb = big.tile([128, HL], F32)
            bvr_sb = big.tile([1, E], BF16)
            onesc_sb = big.tile([128, 1], BF16)
            onesr_sb = big.tile([1, 128], BF16)
            qT_sb = big.tile([128, HL * T], BF16)         # Q^T (scaled, biased)
            kT_sb = big.tile([128, HL * T], BF16)         # K_new^T (biased)
            v_sb = big.tile([128, SCH * B * E], BF16)     # V_new tiles [tok128, t*E+e]
            attnT_sb = big.tile([128, HL * T], BF16)      # attention output^T

            nc.sync.dma_start(wq_sb[:], wq_t.ap())
            # x chunk-wise, spread over engines' DMA queues so the first
            # projection matmuls start early
            dma_engs = (nc.sync, nc.gpsimd, nc.scalar)
            for c in range(NCH_D):
                dma_engs[c % 3].dma_start(x_sb[:, c * T:(c + 1) * T],
                                          x_t.ap()[:, c * T:(c + 1) * T])
            nc.sync.dma_start(wk_sb[:], wk_t.ap())
            nc.sync.dma_start(wv_sb[:], wv_t.ap())
            nc.sync.dma_start(wo_sb[:], wo_t.ap())
            nc.sync.dma_start(bq_sb[:], bq2.ap())
            nc.sync.dma_start(bk_sb[:], bk2.ap())
            nc.sync.dma_start(bvr_sb[:], bv_row.ap())
            nc.sync.dma_start(onesc_sb[:], ones_c.ap())
            nc.sync.dma_start(onesr_sb[:], ones_r.ap())

            # PSUM tags (one pool, 8 banks total): mm512 x2, small x2,
            # sT x2, pv x2
            psum_cm = tc.tile_pool(name="psum", bufs=1, space="PSUM")
            psum = psum_cm.__enter__()

            # ---- phase A: projections ----
            for w_sb, b_sb, dstT, is_k in ((wq_sb, bq_sb, qT_sb, False),
                                           (wk_sb, bk_sb, kT_sb, True)):
                for et in range(HL):
                    for nb in range(T // NB):
                        ps = psum.tile([128, NB], F32, tag="mm512", bufs=2)
                        for c in range(NCH_D):
                            nc.tensor.matmul(
                                ps[:],
                                w_sb[:, c * E + et * 128:c * E + et * 128 + 128],
                                x_sb[:, c * T + nb * NB:c * T + (nb + 1) * NB],
                                start=(c == 0), stop=(c == NCH_D - 1))
                        col = et * T + nb * NB
                        if is_k:
                            kf = work.tile([128, NB], F32, tag="kf32", bufs=2)
                            nc.scalar.activation(kf[:], ps[:], AF.Identity,
                                                 bias=b_sb[:, et:et + 1])
                            nc.vector.tensor_copy(dstT[:, col:col + NB], kf[:])
                            nc.sync.dma_start(
                                k_new_t.ap()[et * 128:(et + 1) * 128,
                                             nb * NB:(nb + 1) * NB], kf[:])
                        else:
                            nc.scalar.activation(dstT[:, col:col + NB], ps[:],
                                                 AF.Identity, bias=b_sb[:, et:et + 1])

            # V: bias seeded by a rank-1 ones @ bv matmul
            for t in range(T // 128):
                ps = psum.tile([128, E], F32, tag="small", bufs=2)
                nc.tensor.matmul(ps[:], onesr_sb[:], bvr_sb[:],
                                 start=True, stop=False)
                for c in range(NCH_D):
                    nc.tensor.matmul(
                        ps[:],
                        x_sb[:, c * T + t * 128:c * T + t * 128 + 128],
                        wv_sb[:, c * E:(c + 1) * E],
                        start=False, stop=(c == NCH_D - 1))
                nc.scalar.activation(v_sb[:, t * E:(t + 1) * E], ps[:], AF.Copy)
                vf = work.tile([128, E], F32, tag="vf32", bufs=2)
                nc.vector.tensor_copy(vf[:], ps[:])
                nc.sync.dma_start(v_new.ap()[t * 128:(t + 1) * 128, :], vf[:])

            # ---- phase B: attention, 8 subunits (batch, head, q-block) ----
            def chunk_aps(b, h, j, pastk, pastv):
                if j < PCH:
                    return (pastk[:, j * 128:(j + 1) * 128],
                            pastv[:, j * 128:(j + 1) * 128])
                jj = j - PCH
                return (kT_sb[:, h * T + b * S + jj * 128:
                              h * T + b * S + jj * 128 + 128],
                        v_sb[:, (b * SCH + jj) * E + h * 128:
                             (b * SCH + jj) * E + h * 128 + 128])

            # batch-0 W_o emitted right after its attention units so PE
            # stays warm across the phase boundary; drains on DVE because
            # ScalarE rate-limits the exp stream still running for batch 1
            def emit_wo(t_lo, t_hi):
                for t in range(t_lo, t_hi):
                    for nb in range(D // NB):
                        ps = psum.tile([128, NB], F32, tag="mm512", bufs=2,
                                       name=f"wo_ps_{t}_{nb}")
                        for h2 in range(HL):
                            nc.tensor.matmul(
                                ps[:],
                                attnT_sb[:, h2 * T + t * 128:h2 * T + t * 128 + 128],
                                wo_sb[:, h2 * D + nb * NB:h2 * D + (nb + 1) * NB],
                                start=(h2 == 0), stop=(h2 == HL - 1))
                        osb = work.tile([128, NB], F32, tag="osb", bufs=4,
                                        name=f"osb_{t}_{nb}")
                        nc.vector.tensor_copy(osb[:], ps[:])
                        nc.sync.dma_start(
                            out_p.ap()[t * 128:(t + 1) * 128,
                                       nb * NB:(nb + 1) * NB],
                            osb[:])

            for u in range(B * HL):
                b, h = u // HL, u % HL
                pastk = work.tile([128, PAST], BF16, tag="pastk", bufs=2)
                nc.sync.dma_start(pastk[:], past_kt.ap()[u])
                pastv = work.tile([128, PAST], BF16, tag="pastv", bufs=2)
                nc.sync.dma_start(pastv[:], past_v.ap()[u])
                for qb in range(QB):
                    q0 = b * S + qb * NB
                    q_ap = qT_sb[:, h * T + q0:h * T + q0 + NB]
                    pv = psum.tile([128, NB], F32, tag="pv", bufs=2)
                    sums = psum.tile([1, NB], F32, tag="small", bufs=2)
                    for j in range(KCH):
                        kT_ap, v_ap = chunk_aps(b, h, j, pastk, pastv)
                        sT = psum.tile([128, NB], F32, tag="sT", bufs=2)
                        nc.tensor.matmul(sT[:], kT_ap, q_ap, start=True, stop=True)
                        pT = work.tile([128, NB], BF16, tag="pT", bufs=4)
                        nc.scalar.activation(pT[:], sT[:], AF.Exp)
                        nc.tensor.matmul(pv[:], v_ap, pT[:],
                                         start=(j == 0), stop=(j == KCH - 1))
                        nc.tensor.matmul(sums[:], onesc_sb[:], pT[:],
                                         start=(j == 0), stop=(j == KCH - 1))
                    # drain accumulators fast; normalize off the critical path
                    araw = work.tile([128, NB], F32, tag="araw", bufs=2)
                    nc.vector.tensor_copy(araw[:], pv[:])
                    recip = work.tile([1, NB], F32, tag="recip", bufs=2)
                    nc.vector.reciprocal(recip[:], sums[:])
                    recip_b = work.tile([128, NB], F32, tag="recip_b", bufs=2)
                    nc.gpsimd.partition_broadcast(recip_b[:], recip[:])
                    nc.vector.tensor_mul(attnT_sb[:, h * T + q0:h * T + q0 + NB],
                                         araw[:], recip_b[:])
                if u == HL - 1:
                    emit_wo(0, T // 256)

            # ---- phase C: output projection, batch-1 tail ----
            for t in range(T // 256, T // 128):
                for nb in range(D // NB):
                    ps = psum.tile([128, NB], F32, tag="mm512", bufs=2)
                    for h in range(HL):
                        nc.tensor.matmul(
                            ps[:],
                            attnT_sb[:, h * T + t * 128:h * T + t * 128 + 128],
                            wo_sb[:, h * D + nb * NB:h * D + (nb + 1) * NB],
                            start=(h == 0), stop=(h == HL - 1))
                    osb = work.tile([128, NB], F32, tag="osb", bufs=4)
                    nc.scalar.copy(osb[:], ps[:])
                    nc.sync.dma_start(
                        out_p.ap()[t * 128:(t + 1) * 128, nb * NB:(nb + 1) * NB],
                        osb[:])
            psum_cm.__exit__(None, None, None)

    nc.compile()
    return nc


def _prep_inputs(x, past_key, past_value, Wq, bq, Wk, bk, Wv, bv, Wo, bo):
    """Build the 8 per-core input maps (all arrays pre-arranged to SBUF layouts)."""
    scl = np.float32(1.0 / np.sqrt(DK))
    xf = np.asarray(x, np.float32).reshape(T, D)
    # x^T chunks: [p, c*T + t] = x[t, c*128+p]
    x_t = np.ascontiguousarray(xf.reshape(T, NCH_D, 128).transpose(2, 1, 0)
                               ).reshape(128, NCH_D * T).astype(_BF)
    ones_c = np.ones((128, 1), _BF)
    ones_r = np.ones((1, 128), _BF)

    def w_chunks(Wslice):  # [E, D] -> W^T chunk layout [128, NCH_D*E]
        wt = np.asarray(Wslice, np.float32).T  # [D, E]
        return np.ascontiguousarray(wt.reshape(NCH_D, 128, E).transpose(1, 0, 2)
                                    ).reshape(128, NCH_D * E).astype(_BF)

    in_maps = []
    for c in range(NCORES):
        r0, r1 = c * E, (c + 1) * E
        wo_c = np.asarray(Wo, np.float32)[:, r0:r1].T  # [E, D]
        wo_t = np.ascontiguousarray(wo_c.reshape(HL, 128, D).transpose(1, 0, 2)
                                    ).reshape(128, HL * D).astype(_BF)
        pk = np.asarray(past_key, np.float32)[:, c * HL:(c + 1) * HL]   # [B,HL,PAST,DK]
        pkt = np.ascontiguousarray(pk.transpose(0, 1, 3, 2)
                                   ).reshape(B * HL, 128, PAST).astype(_BF)
        pvv = np.asarray(past_value, np.float32)[:, c * HL:(c + 1) * HL]
        pvt = np.ascontiguousarray(
            pvv.reshape(B, HL, PCH, 128, DK).transpose(0, 1, 3, 2, 4)
        ).reshape(B * HL, 128, PAST).astype(_BF)
        in_maps.append({
            "x_t": x_t,
            "wq_t": w_chunks(np.asarray(Wq, np.float32)[r0:r1] * scl),
            "wk_t": w_chunks(np.asarray(Wk)[r0:r1]),
            "wv_t": w_chunks(np.asarray(Wv)[r0:r1]),
            "wo_t": wo_t,
            "bq2": np.ascontiguousarray(
                (np.asarray(bq, np.float32)[r0:r1] * scl).reshape(HL, 128).T),
            "bk2": np.ascontiguousarray(
                np.asarray(bk, np.float32)[r0:r1].reshape(HL, 128).T),
            "bv_row": np.asarray(bv, np.float32)[r0:r1].reshape(1, E).astype(_BF),
            "ones_c": ones_c,
            "ones_r": ones_r,
            "past_kt": pkt,
            "past_v": pvt,
        })
    return in_maps


def _assemble(results, past_key, past_value, bo):
    out = np.zeros((T, D), np.float32)
    for c in range(NCORES):
        out += results[c]["out_p"]
    out += np.asarray(bo, np.float32)
    out = out.reshape(B, S, D)

    K = np.empty((B, H, PAST + S, DK), np.float32)
    V = np.empty((B, H, PAST + S, DK), np.float32)
    K[:, :, :PAST] = np.asarray(past_key, np.float32)
    V[:, :, :PAST] = np.asarray(past_value, np.float32)
    for c in range(NCORES):
        knt = results[c]["k_new_t"].reshape(HL, 128, B, S).transpose(2, 0, 3, 1)
        K[:, c * HL:(c + 1) * HL, PAST:] = knt
        vnw = results[c]["v_new"].reshape(B, S, HL, DK).transpose(0, 2, 1, 3)
        V[:, c * HL:(c + 1) * HL, PAST:] = vnw
    return out, K, V


def run(inputs, trace=False):
    if "nc" not in _CACHE:
        _CACHE["nc"] = _build()
    nc = _CACHE["nc"]
    in_maps = _prep_inputs(**inputs)
    res = run_bass_kernel_spmd(nc, in_maps, core_ids=list(range(NCORES)),
                               trace=trace)
    out, K, V = _assemble(res.results, inputs["past_key"], inputs["past_value"],
                          inputs["bo"])
    return (out, K, V), res


def kernel(**inputs):
    (out, K, V), _ = run(inputs)
    return out, K, V
